# revision 1
# baseline (speedup 1.0000x reference)
"""Trainium2 Bass kernel for nn_DeepCluster (vq_codebook).

Computation (per row x of shape [72]):
  7-layer MLP (Linear chain, ReLU after layers 2 and 4) -> f [200]
  sq[j]  = |f|^2 - 2*(f @ center)[j] + |center[:, j]|^2      (center: [200, 72])
  nom    = 1 / (1 + sq)                                       (alpha = 1)
  q      = nom / sum_j nom

Strategy: pure data parallel over 8 NeuronCores (batch split).  On each
core, activations flow as [features(partitions), batch(free)] bf16 tiles
of 512 rows; bf16 matmuls stream at 1 cycle/row on the PE with fast
weight load.  The distance computation runs transposed ([cluster,
batch]) so its matmuls also get a 512-wide moving operand; |c_j|^2 + 1
is added per-partition in f32 (it dominates sq, so it must not be
rounded to bf16), and the value path after the reciprocal stays f32.
PSUM->SBUF epilogues (bias add + optional ReLU) are split between the
Scalar and Vector engines.  The per-tile tail (reciprocal -> transpose
back -> row-normalize -> store) is software-pipelined one tile behind
the matmul stage so the PE never waits on the DVE round trip.
"""

import numpy as np

DIMS = [72, 128, 256, 256, 512, 512, 512, 200]
RELU_LAYERS = {2, 4}  # 1-indexed layers followed by ReLU
N_CORES = 8
N_FULL = 262144
B = 512  # rows per pipeline tile
P = 128

_CACHE = {}


def _build(n_rows):
    import concourse.bass as bass
    import concourse.mybir as mybir
    from concourse import bacc
    from concourse.tile import TileContext
    from concourse.masks import make_identity

    f32 = mybir.dt.float32
    bf16 = mybir.dt.bfloat16
    AF = mybir.ActivationFunctionType
    AX = mybir.AxisListType
    ALU = mybir.AluOpType

    kc_l = [(DIMS[i] + 127) // 128 for i in range(7)]
    mc_l = [(DIMS[i + 1] + 127) // 128 for i in range(7)]

    nc = bacc.Bacc(None, target_bir_lowering=False, debug=False)
    x_d = nc.dram_tensor("x", [n_rows, 72], bf16, kind="ExternalInput")
    q_d = nc.dram_tensor("q", [n_rows, 72], f32, kind="ExternalOutput")
    w_d, b_d = [], []
    for l in range(7):
        din, dout = DIMS[l], DIMS[l + 1]
        w_d.append(
            nc.dram_tensor(
                f"w{l + 1}", [min(din, 128), kc_l[l] * dout], bf16, kind="ExternalInput"
            )
        )
        b_d.append(nc.dram_tensor(f"b{l + 1}", [128, mc_l[l]], f32, kind="ExternalInput"))
    cm2A_d = nc.dram_tensor("cm2A", [128, 72], bf16, kind="ExternalInput")
    cm2B_d = nc.dram_tensor("cm2B", [72, 72], bf16, kind="ExternalInput")
    csq1_d = nc.dram_tensor("csq1", [72, 1], f32, kind="ExternalInput")

    n_tiles = n_rows // B
    assert n_rows % B == 0
    C = B // P  # 128-row chunks per tile

    with TileContext(nc) as tc:
        with (
            tc.tile_pool(name="consts", bufs=1) as consts,
            tc.tile_pool(name="acts", bufs=3) as acts,
            tc.tile_pool(name="pmm", bufs=4, space="PSUM") as pmm,
            tc.tile_pool(name="ptp", bufs=1, space="PSUM") as ptp,
            tc.tile_pool(name="psd", bufs=2, space="PSUM") as psd,
            tc.tile_pool(name="ppq", bufs=1, space="PSUM") as ppq,
        ):
            ones = consts.tile([128, 72], bf16, tag="ones")
            nc.vector.memset(ones, 1.0)
            ident = consts.tile([128, 128], bf16, tag="ident")
            make_identity(nc, ident)
            identf = consts.tile([128, 128], f32, tag="identf")
            make_identity(nc, identf)
            cm2A = consts.tile([128, 72], bf16, tag="cm2A")
            nc.sync.dma_start(out=cm2A, in_=cm2A_d[:])
            cm2B = consts.tile([72, 72], bf16, tag="cm2B")
            nc.sync.dma_start(out=cm2B, in_=cm2B_d[:])
            csq1 = consts.tile([72, 1], f32, tag="csq1")
            nc.sync.dma_start(out=csq1, in_=csq1_d[:])
            w_sb, b_sb = [], []
            for l in range(7):
                wt = consts.tile(list(w_d[l].shape), bf16, tag=f"w{l}")
                nc.sync.dma_start(out=wt, in_=w_d[l][:])
                w_sb.append(wt)
                bt = consts.tile([128, mc_l[l]], f32, tag=f"bias{l}")
                nc.sync.dma_start(out=bt, in_=b_d[l][:])
                b_sb.append(bt)

            x_r = x_d[:].rearrange("(t c p) j -> t p c j", p=P, c=C)
            q_r = q_d[:].rearrange("(t s p) j -> t p s j", p=P, s=C)

            def stageX(t):
                """x load + transpose -> xT [72, B] bf16 in SBUF."""
                x_sb = acts.tile([P, C, 72], bf16, tag="x")
                nc.sync.dma_start(out=x_sb, in_=x_r[t])
                ptx = ptp.tile([72, B], bf16, tag="xtp")
                for c in range(C):
                    nc.tensor.transpose(
                        ptx[:, P * c : P * (c + 1)], x_sb[:, c, :], ident
                    )
                xT = acts.tile([72, B], bf16, tag="xT")
                nc.vector.tensor_copy(xT, ptx)
                return xT

            def stageM(t, xT, next_xT_cb):
                """MLP + g + distance matmuls -> sdT PSUM.  Emits the next
                tile's input transposes mid-chain so the PE has filler work
                at layer-boundary epilogue stalls."""
                h = [xT]
                ep = 0
                for l in range(7):
                    dout = DIMS[l + 1]
                    kc, mc = kc_l[l], mc_l[l]
                    relu = (l + 1) in RELU_LAYERS
                    hn = []
                    for m in range(mc):
                        pw = min(128, dout - 128 * m)
                        ps = pmm.tile([pw, B], f32, tag="mm")
                        for k in range(kc):
                            lhsT = w_sb[l][:, k * dout + 128 * m : k * dout + 128 * m + pw]
                            nc.tensor.matmul(
                                ps, lhsT, h[k], start=(k == 0), stop=(k == kc - 1)
                            )
                        ht = acts.tile([pw, B], bf16, tag=f"h{l + 1}m{m}")
                        bias_col = b_sb[l][:pw, m : m + 1]
                        if ep % 2 == 0:  # scalar engine (ACT)
                            nc.scalar.activation(
                                out=ht,
                                in_=ps,
                                func=AF.Relu if relu else AF.Identity,
                                bias=bias_col,
                                scale=1.0,
                            )
                        else:  # vector engine (DVE)
                            if relu:
                                nc.vector.tensor_scalar(
                                    out=ht,
                                    in0=ps,
                                    scalar1=bias_col,
                                    scalar2=0.0,
                                    op0=ALU.add,
                                    op1=ALU.max,
                                )
                            else:
                                nc.vector.tensor_scalar_add(ht, ps, bias_col)
                        ep += 1
                        hn.append(ht)
                    h = hn
                    if l == 1 and next_xT_cb is not None:
                        next_xT_cb()

                f0, f1 = h  # [128, B], [72, B] bf16
                g0 = acts.tile([128, B], bf16, tag="g0")
                nc.vector.tensor_mul(g0, f0, f0)
                g1 = acts.tile([72, B], bf16, tag="g1")
                nc.vector.tensor_mul(g1, f1, f1)

                sdT = psd.tile([72, B], f32, tag="sd")
                nc.tensor.matmul(sdT, ones[:128, :72], g0, start=True, stop=False)
                nc.tensor.matmul(sdT, ones[:72, :72], g1, start=False, stop=False)
                nc.tensor.matmul(sdT, cm2A, f0, start=False, stop=False)
                nc.tensor.matmul(sdT, cm2B, f1, start=False, stop=True)
                return sdT

            def stageB(t, sdT):
                """csq add + reciprocal + transpose back + normalize + store."""
                sd1 = acts.tile([72, B], f32, tag="sd1")
                nc.scalar.activation(
                    out=sd1, in_=sdT, func=AF.Identity, bias=csq1[:, 0:1], scale=1.0
                )
                nomT = acts.tile([72, B], f32, tag="nomT")
                nc.vector.reciprocal_approx_fast(out=nomT, in_=sd1)

                pq = ppq.tile([P, C, 72], f32, tag="pq")
                for s in range(C):
                    nc.tensor.transpose(
                        pq[:, s, :], nomT[:, P * s : P * (s + 1)], identf[:72, :72]
                    )
                rs4 = acts.tile([P, C], f32, tag="rs4")
                nc.vector.reduce_sum(rs4, pq, axis=AX.X)
                rr4 = acts.tile([P, C], f32, tag="rr4")
                nc.vector.reciprocal(rr4, rs4)
                rr_b = bass.AP(
                    tensor=rr4.tensor,
                    offset=rr4.offset,
                    ap=[rr4.ap[0], rr4.ap[1], [0, 72]],
                )
                qt = acts.tile([P, C, 72], f32, tag="qt")
                nc.vector.tensor_tensor(out=qt, in0=pq, in1=rr_b, op=ALU.mult)
                nc.sync.dma_start(out=q_r[t], in_=qt)

            prev = None
            next_xT = [stageX(0)]

            for t in range(n_tiles):

                def make_cb(tn):
                    if tn >= n_tiles:
                        return None

                    def cb():
                        next_xT.append(stageX(tn))

                    return cb

                cur = (t, stageM(t, next_xT.pop(0), make_cb(t + 1)))
                if prev is not None:
                    stageB(*prev)
                prev = cur
            stageB(*prev)

    nc.compile()
    return nc


def _prep_consts(ws, bs, center):
    """Host-side marshalling of the small replicated weights."""
    import ml_dtypes

    bf = ml_dtypes.bfloat16
    kc_l = [(DIMS[i] + 127) // 128 for i in range(7)]
    mc_l = [(DIMS[i + 1] + 127) // 128 for i in range(7)]
    consts = {}
    for l in range(7):
        din, dout = DIMS[l], DIMS[l + 1]
        w = np.ascontiguousarray(ws[l], dtype=np.float32)
        if din > 128:
            kc = kc_l[l]
            w = np.ascontiguousarray(
                w.reshape(kc, 128, dout).transpose(1, 0, 2).reshape(128, kc * dout)
            )
        consts[f"w{l + 1}"] = w.astype(bf)
        bt = np.zeros((128, mc_l[l]), dtype=np.float32)
        for m in range(mc_l[l]):
            pw = min(128, dout - 128 * m)
            bt[:pw, m] = bs[l][128 * m : 128 * m + pw]
        consts[f"b{l + 1}"] = bt
    c = np.asarray(center, dtype=np.float32)
    consts["cm2A"] = np.ascontiguousarray(-2.0 * c[:128, :]).astype(bf)
    consts["cm2B"] = np.ascontiguousarray(-2.0 * c[128:, :]).astype(bf)
    consts["csq1"] = np.ascontiguousarray(
        (1.0 + (c.astype(np.float64) ** 2).sum(axis=0)).reshape(72, 1)
    ).astype(np.float32)
    return consts


def kernel(
    inputs, w1, b1, w2, b2, w3, b3, w4, b4, w5, b5, w6, b6, w7, b7, center
):
    import ml_dtypes
    from concourse.bass_utils import run_bass_kernel_spmd

    x = np.asarray(inputs).astype(ml_dtypes.bfloat16)
    n = x.shape[0]
    n_loc = n // N_CORES
    key = n_loc
    if key not in _CACHE:
        _CACHE[key] = _build(n_loc)
    nc = _CACHE[key]

    consts = _prep_consts(
        [w1, w2, w3, w4, w5, w6, w7], [b1, b2, b3, b4, b5, b6, b7], center
    )
    in_maps = []
    for c in range(N_CORES):
        m = {"x": np.ascontiguousarray(x[c * n_loc : (c + 1) * n_loc])}
        m.update(consts)
        in_maps.append(m)
    res = run_bass_kernel_spmd(nc, in_maps, core_ids=list(range(N_CORES)))
    return np.concatenate([res.results[c]["q"] for c in range(N_CORES)], axis=0)



# revision 13
# speedup vs baseline: 1.8431x; 1.8431x over previous
"""Trainium2 Bass kernel for nn_DeepCluster (vq_codebook).

Computation (per row x of shape [72]):
  7-layer MLP (Linear chain, ReLU after layers 2 and 4) -> f [200]
  sq[j]  = |f|^2 - 2*(f @ center)[j] + |center[:, j]|^2      (center: [200, 72])
  nom    = 1 / (1 + sq)                                       (alpha = 1)
  q      = nom / sum_j nom

Strategy (pure data parallel over 8 cores, batch split, B=512 row tiles):

* ReLU only follows layers 2 and 4, so the linear chains compose exactly
  (in float64 on the host):
     LA = L1*L2    [72 -> 256]   (+ bias folded via a ones-row of xT)
     LB = L3*L4    [256 -> 512]
     LC = L5*L6*L7 [512 -> 200]
  This cuts PE work ~3x and halves the PSUM->SBUF epilogue traffic,
  which only ACT and DVE can carry on trn2 (GPSIMD has no PSUM port).
* LB / LC run as fp8e4 DoubleRow matmuls (two 128-row k-tiles per
  instruction, 2x PE throughput).  Activations carry per-layer
  power-of-two scales (beta = 2 / 8 / 64) chosen so every epilogue is
  scale-free; the scales are folded into the host-marshalled weights
  and biases, so epilogues are single Relu / bias-add ops split across
  the ACT and DVE engines.
* The distance stage runs transposed ([cluster, batch]) in bf16; the
  "+1 + |c_j|^2" constant is folded into the squares matmul via two
  spare partition rows of f (preset to 1.0) against a value+residual
  pair of bf16 weight rows, so sdT leaves PSUM complete and feeds the
  DVE reciprocal directly.
* x is transposed to [72, N] on the host (free), so the input path has
  no PE transposes or PSUM round trip.  q is produced in bf16 and
  upcast to f32 on the host.
"""

import numpy as np

N_CORES = 8
B = 512  # rows per pipeline tile
P = 128

_CACHE = {}


def _build(n_rows):
    import concourse.bass as bass
    import concourse.mybir as mybir
    from concourse import bacc
    from concourse.tile import TileContext
    from concourse.masks import make_identity

    f32 = mybir.dt.float32
    bf16 = mybir.dt.bfloat16
    fp8 = mybir.dt.float8e4
    AF = mybir.ActivationFunctionType
    AX = mybir.AxisListType
    ALU = mybir.AluOpType
    DR = mybir.MatmulPerfMode.DoubleRow

    nc = bacc.Bacc(None, target_bir_lowering=False, debug=False)
    xT_d = nc.dram_tensor("xT", [72, n_rows], bf16, kind="ExternalInput")
    q_d = nc.dram_tensor("q", [n_rows, 72], bf16, kind="ExternalOutput")
    w12_d = nc.dram_tensor("w12", [73, 256], bf16, kind="ExternalInput")
    W34_d = nc.dram_tensor("W34", [128, 2 * 512], fp8, kind="ExternalInput")
    # dout padded 200 -> 256 so the k-pair AP step is 16-byte aligned
    # (walrus `s3_lw_dual_fp8_restrictions` requires step % 16 == 0)
    W567_d = nc.dram_tensor("W567", [128, 4 * 256], fp8, kind="ExternalInput")
    b34_d = nc.dram_tensor("b34", [128, 4], f32, kind="ExternalInput")
    b567_d = nc.dram_tensor("b567", [128, 2], f32, kind="ExternalInput")
    sqA_d = nc.dram_tensor("sqA", [128, 72], bf16, kind="ExternalInput")
    sqB_d = nc.dram_tensor("sqB", [128, 72], bf16, kind="ExternalInput")
    crA_d = nc.dram_tensor("crA", [128, 72], bf16, kind="ExternalInput")
    crB_d = nc.dram_tensor("crB", [128, 72], bf16, kind="ExternalInput")

    n_tiles = n_rows // B
    assert n_rows % B == 0
    C = B // P  # 128-row chunks per tile

    with TileContext(nc) as tc:
        with (
            tc.tile_pool(name="consts", bufs=1) as consts,
            tc.tile_pool(name="acts", bufs=2) as acts,
            tc.tile_pool(name="pmm", bufs=2, space="PSUM") as pmm,
            tc.tile_pool(name="psd", bufs=2, space="PSUM") as psd,
            tc.tile_pool(name="ppq", bufs=2, space="PSUM") as ppq,
        ):
            identf = consts.tile([128, 128], f32, tag="identf")
            make_identity(nc, identf)
            w12 = consts.tile([73, 256], bf16, tag="w12")
            nc.sync.dma_start(out=w12, in_=w12_d[:])
            W34 = consts.tile([128, 2, 512], fp8, tag="W34")
            nc.sync.dma_start(out=W34, in_=W34_d[:].rearrange("p (k o) -> p k o", k=2))
            W567 = consts.tile([128, 4, 256], fp8, tag="W567")
            nc.sync.dma_start(
                out=W567, in_=W567_d[:].rearrange("p (k o) -> p k o", k=4)
            )
            b34 = consts.tile([128, 4], f32, tag="b34")
            nc.sync.dma_start(out=b34, in_=b34_d[:])
            b567 = consts.tile([128, 2], f32, tag="b567")
            nc.sync.dma_start(out=b567, in_=b567_d[:])
            sqA = consts.tile([128, 72], bf16, tag="sqA")
            nc.sync.dma_start(out=sqA, in_=sqA_d[:])
            sqB = consts.tile([128, 72], bf16, tag="sqB")
            nc.sync.dma_start(out=sqB, in_=sqB_d[:])
            crA = consts.tile([128, 72], bf16, tag="crA")
            nc.sync.dma_start(out=crA, in_=crA_d[:])
            crB = consts.tile([128, 72], bf16, tag="crB")
            nc.sync.dma_start(out=crB, in_=crB_d[:])

            # xT ring: row 72 is a constant ones-row (bias input for LA).
            NXT = 3
            xt_bufs = []
            for i in range(NXT):
                t_ = consts.tile([73, B], bf16, tag=f"xT{i}")
                # rows 64:72 are re-written by the per-tile DMA; row 72
                # stays 1.0 (engine ops must start at a multiple of 32)
                nc.vector.memset(t_[64:73, :], 1.0)
                xt_bufs.append(t_)
            # f ring: rows 72:74 of chunk 1 are ones-rows that pick up the
            # (1 + |c_j|^2) value+residual weight rows; 74:128 stay zero.
            NF = 2
            f_bufs = []
            for i in range(NF):
                t_ = consts.tile([128, 2, B], bf16, tag=f"f{i}")
                # rows 64:72 are re-written by the per-tile L7m1 epilogue
                nc.vector.memset(t_[64:128, 1, :], 0.0)
                nc.vector.memset(t_[64:74, 1, :], 1.0)
                f_bufs.append(t_)

            xT_r = xT_d[:].rearrange("j (t b) -> t j b", b=B)
            q_r = q_d[:].rearrange("(t s p) j -> t p s j", p=P, s=C)

            def dma_in(t):
                nc.sync.dma_start(out=xt_bufs[t % NXT][0:72, :], in_=xT_r[t])

            fg = {}    # t -> (f, g)
            nomT = {}  # t -> nomT tile
            pqs = {}   # t -> pq psum tile

            def do_dist(t):
                """dist matmuls + reciprocal for tile t."""
                f, g = fg.pop(t)
                sdT = psd.tile([72, B], f32, tag="sd")
                nc.tensor.matmul(sdT, sqA, g[:, 0, :], start=True, stop=False)
                nc.tensor.matmul(sdT, sqB, g[:, 1, :], start=False, stop=False)
                nc.tensor.matmul(sdT, crA, f[:, 0, :], start=False, stop=False)
                nc.tensor.matmul(sdT, crB, f[:, 1, :], start=False, stop=True)
                nt = acts.tile([72, B], f32, tag="nomT")
                nc.vector.reciprocal_approx_fast(out=nt, in_=sdT)
                nomT[t] = nt

            def do_pq(t):
                """transpose nomT(t) back to row-major (on PE)."""
                nt = nomT.pop(t)
                pq = ppq.tile([P, C, 72], f32, tag="pq")
                for s in range(C):
                    nc.tensor.transpose(
                        pq[:, s, :], nt[:, P * s : P * (s + 1)], identf[:72, :72]
                    )
                pqs[t] = pq

            def stage3(t):
                """reduce + normalize + store for tile t."""
                pq = pqs.pop(t)
                rs4 = acts.tile([P, C], f32, tag="rs4")
                nc.vector.reduce_sum(rs4, pq, axis=AX.X)
                rr4 = acts.tile([P, C], f32, tag="rr4")
                nc.vector.reciprocal(rr4, rs4)
                rr_b = bass.AP(
                    tensor=rr4.tensor,
                    offset=rr4.offset,
                    ap=[rr4.ap[0], rr4.ap[1], [0, 72]],
                )
                qt = acts.tile([P, C, 72], bf16, tag="qt")
                nc.vector.tensor_tensor(out=qt, in0=pq, in1=rr_b, op=ALU.mult)
                nc.sync.dma_start(out=q_r[t], in_=qt)

            def stage1(t, mid1, mid2):
                """LA -> LB -> LC matmuls + epilogues -> f', g.

                mid1/mid2 are PE filler callbacks (dist(t-1), pq(t-2))
                emitted between layer boundaries so the PE keeps busy
                while ACT/DVE drain the previous layer's PSUM."""
                xt = xt_bufs[t % NXT]
                psA = pmm.tile([128, 2, B], f32, tag="mm")
                nc.tensor.matmul(
                    psA[:, 0, :], w12[:, 0:128], xt, start=True, stop=True
                )
                nc.tensor.matmul(
                    psA[:, 1, :], w12[:, 128:256], xt, start=True, stop=True
                )
                if mid1 is not None:
                    mid1()
                hA = acts.tile([128, 2, B], fp8, tag="hA")
                nc.scalar.activation(out=hA[:, 0, :], in_=psA[:, 0, :], func=AF.Relu)
                nc.vector.tensor_scalar_max(hA[:, 1, :], psA[:, 1, :], 0.0)

                psB = pmm.tile([128, 2, B], f32, tag="mm")
                for m in range(2):
                    nc.tensor.matmul(
                        psB[:, m, :],
                        W34[:, 0:2, 128 * m : 128 * (m + 1)],
                        hA[:, 0:2, :],
                        start=True,
                        stop=True,
                        perf_mode=DR,
                    )
                if mid2 is not None:
                    mid2()
                psB2 = pmm.tile([128, 2, B], f32, tag="mm")
                for m in range(2, 4):
                    nc.tensor.matmul(
                        psB2[:, m - 2, :],
                        W34[:, 0:2, 128 * m : 128 * (m + 1)],
                        hA[:, 0:2, :],
                        start=True,
                        stop=True,
                        perf_mode=DR,
                    )
                hB = acts.tile([128, 4, B], fp8, tag="hB")
                nc.scalar.activation(
                    out=hB[:, 0, :], in_=psB[:, 0, :], func=AF.Relu,
                    bias=b34[:, 0:1],
                )
                nc.vector.tensor_scalar(
                    out=hB[:, 1, :], in0=psB[:, 1, :],
                    scalar1=b34[:, 1:2], scalar2=0.0, op0=ALU.add, op1=ALU.max,
                )
                nc.scalar.activation(
                    out=hB[:, 2, :], in_=psB2[:, 0, :], func=AF.Relu,
                    bias=b34[:, 2:3],
                )
                nc.vector.tensor_scalar(
                    out=hB[:, 3, :], in0=psB2[:, 1, :],
                    scalar1=b34[:, 3:4], scalar2=0.0, op0=ALU.add, op1=ALU.max,
                )

                psF = pmm.tile([128, 2, B], f32, tag="mm")
                nc.tensor.matmul(
                    psF[:, 0, :], W567[:, 0:2, 0:128], hB[:, 0:2, :],
                    start=True, stop=False, perf_mode=DR,
                )
                nc.tensor.matmul(
                    psF[:, 0, :], W567[:, 2:4, 0:128], hB[:, 2:4, :],
                    start=False, stop=True, perf_mode=DR,
                )
                nc.tensor.matmul(
                    psF[:, 1, :], W567[:, 0:2, 128:256], hB[:, 0:2, :],
                    start=True, stop=False, perf_mode=DR,
                )
                nc.tensor.matmul(
                    psF[:, 1, :], W567[:, 2:4, 128:256], hB[:, 2:4, :],
                    start=False, stop=True, perf_mode=DR,
                )
                f = f_bufs[t % NF]
                nc.vector.tensor_scalar_add(f[:, 0, :], psF[:, 0, :], b567[:, 0:1])
                nc.scalar.activation(
                    out=f[0:72, 1, :], in_=psF[0:72, 1, :], func=AF.Identity,
                    bias=b567[0:72, 1:2],
                )
                g = acts.tile([128, 2, B], bf16, tag="g")
                nc.scalar.square(g, f)
                fg[t] = (f, g)

            dma_in(0)
            if n_tiles > 1:
                dma_in(1)
            for t in range(n_tiles):
                if t + 2 < n_tiles:
                    dma_in(t + 2)
                mid1 = (lambda tt: lambda: do_dist(tt))(t - 1) if t >= 1 else None
                mid2 = (lambda tt: lambda: do_pq(tt))(t - 2) if t >= 2 else None
                stage1(t, mid1, mid2)
                if t >= 2:
                    stage3(t - 2)
            # drain
            do_dist(n_tiles - 1)
            if n_tiles >= 2:
                do_pq(n_tiles - 2)
                stage3(n_tiles - 2)
            do_pq(n_tiles - 1)
            stage3(n_tiles - 1)

    nc.compile()
    return nc


def _prep_consts(ws, bs, center):
    """Host-side composition + marshalling of the replicated weights."""
    import ml_dtypes

    bf = ml_dtypes.bfloat16
    f8 = ml_dtypes.float8_e4m3fn
    w = [np.asarray(x, dtype=np.float64) for x in ws]
    b = [np.asarray(x, dtype=np.float64) for x in bs]
    c = np.asarray(center, dtype=np.float64)

    w12 = w[0] @ w[1]
    b12 = b[0] @ w[1] + b[1]
    W34 = w[2] @ w[3]
    b34 = b[2] @ w[3] + b[3]
    W567 = w[4] @ w[5] @ w[6]
    b567 = (b[4] @ w[5] + b[5]) @ w[6] + b[6]

    consts = {}
    consts["w12"] = np.ascontiguousarray(
        2.0 * np.vstack([w12, b12[None, :]])
    ).astype(bf)
    # k-major fp8 layouts: W[p, k*dout + o] = scale * W[k*128 + p, o]
    consts["W34"] = np.ascontiguousarray(
        (4.0 * W34).reshape(2, 128, 512).transpose(1, 0, 2).reshape(128, 1024)
    ).astype(f8)
    W567p = np.concatenate([8.0 * W567, np.zeros((512, 56))], axis=1)  # pad 200->256
    consts["W567"] = np.ascontiguousarray(
        W567p.reshape(4, 128, 256).transpose(1, 0, 2).reshape(128, 1024)
    ).astype(f8)
    consts["b34"] = np.ascontiguousarray(
        (8.0 * b34).reshape(4, 128).T
    ).astype(np.float32)
    bt = np.zeros((128, 2))
    bt[0:128, 0] = 64.0 * b567[0:128]
    bt[0:72, 1] = 64.0 * b567[128:200]
    consts["b567"] = bt.astype(np.float32)

    csq = 1.0 + (c**2).sum(axis=0)  # [72]
    main = csq.astype(bf)
    resid = (csq - main.astype(np.float64)).astype(bf)
    ones_w = np.float64(2.0**-12)  # f' = 64 f, so sum(g')*2^-12 = |f|^2
    sqA = np.full((128, 72), ones_w)
    sqB = np.zeros((128, 72))
    sqB[0:72, :] = ones_w
    sqB[72, :] = main.astype(np.float64)
    sqB[73, :] = resid.astype(np.float64)
    consts["sqA"] = sqA.astype(bf)
    consts["sqB"] = sqB.astype(bf)
    crA = np.ascontiguousarray(-c[0:128, :] / 32.0)  # f' * (-c/32) = -2 f c
    crB = np.zeros((128, 72))
    crB[0:72, :] = -c[128:200, :] / 32.0
    consts["crA"] = crA.astype(bf)
    consts["crB"] = crB.astype(bf)
    return consts


def _make_in_maps(inputs, ws, bs, center):
    import ml_dtypes

    bf = ml_dtypes.bfloat16
    x = np.asarray(inputs)
    n = x.shape[0]
    n_loc = n // N_CORES
    consts = _prep_consts(ws, bs, center)
    xT = np.ascontiguousarray(x.astype(bf).T)  # [72, N]
    in_maps = []
    for cid in range(N_CORES):
        m = {"xT": np.ascontiguousarray(xT[:, cid * n_loc : (cid + 1) * n_loc])}
        m.update(consts)
        in_maps.append(m)
    return in_maps, n_loc


def kernel(
    inputs, w1, b1, w2, b2, w3, b3, w4, b4, w5, b5, w6, b6, w7, b7, center
):
    from concourse.bass_utils import run_bass_kernel_spmd

    in_maps, n_loc = _make_in_maps(
        inputs,
        [w1, w2, w3, w4, w5, w6, w7],
        [b1, b2, b3, b4, b5, b6, b7],
        center,
    )
    if n_loc not in _CACHE:
        _CACHE[n_loc] = _build(n_loc)
    nc = _CACHE[n_loc]
    res = run_bass_kernel_spmd(nc, in_maps, core_ids=list(range(N_CORES)))
    out = np.concatenate(
        [np.asarray(res.results[c]["q"]) for c in range(N_CORES)], axis=0
    )
    return out.astype(np.float32)


# revision 16
# speedup vs baseline: 2.7599x; 1.4974x over previous
"""Trainium2 Bass kernel for nn_DeepCluster (vq_codebook).

Computation (per row x of shape [72]):
  7-layer MLP (Linear chain, ReLU after layers 2 and 4) -> f [200]
  sq[j]  = |f|^2 - 2*(f @ center)[j] + |center[:, j]|^2      (center: [200, 72])
  nom    = 1 / (1 + sq)                                       (alpha = 1)
  q      = nom / sum_j nom

Strategy (pure data parallel over 8 cores, batch split, B=512 row tiles):

* ReLU only follows layers 2 and 4, so the linear chains compose exactly
  (in float64 on the host):
     LA = L1*L2    [72 -> 256]   (+ bias folded via a ones-row of xT)
     LB = L3*L4    [256 -> 512]
     LC = L5*L6*L7 [512 -> 200]
  This cuts PE work ~3x and halves the PSUM->SBUF epilogue traffic,
  which only ACT and DVE can carry on trn2 (GPSIMD has no PSUM port).
* LB / LC run as fp8e4 DoubleRow matmuls (two 128-row k-tiles per
  instruction, 2x PE throughput).  Activations carry per-layer
  power-of-two scales (beta = 2 / 8 / 64) chosen so every epilogue is
  scale-free; the scales are folded into the host-marshalled weights
  and biases, so epilogues are single Relu / bias-add ops split across
  the ACT and DVE engines.
* The distance stage runs transposed ([cluster, batch]) in bf16; the
  "+1 + |c_j|^2" constant is folded into the squares matmul via two
  spare partition rows of f (preset to 1.0) against a value+residual
  pair of bf16 weight rows, so sdT leaves PSUM complete and feeds the
  DVE reciprocal directly.
* x is transposed to [72, N] on the host (free), so the input path has
  no PE transposes or PSUM round trip.  q is produced in bf16 and
  upcast to f32 on the host.
"""

import numpy as np

N_CORES = 8
B = 512  # rows per pipeline tile
P = 128

_CACHE = {}


def _build(n_rows):
    import concourse.bass as bass
    import concourse.mybir as mybir
    from concourse import bacc
    from concourse.tile import TileContext
    from concourse.masks import make_identity

    f32 = mybir.dt.float32
    bf16 = mybir.dt.bfloat16
    fp8 = mybir.dt.float8e4
    AF = mybir.ActivationFunctionType
    AX = mybir.AxisListType
    ALU = mybir.AluOpType
    DR = mybir.MatmulPerfMode.DoubleRow

    nc = bacc.Bacc(None, target_bir_lowering=False, debug=False)
    xT_d = nc.dram_tensor("xT", [72, n_rows], bf16, kind="ExternalInput")
    q_d = nc.dram_tensor("q", [n_rows, 72], bf16, kind="ExternalOutput")
    w12_d = nc.dram_tensor("w12", [73, 256], bf16, kind="ExternalInput")
    W34_d = nc.dram_tensor("W34", [128, 2 * 512], fp8, kind="ExternalInput")
    # dout padded 200 -> 256 so the k-pair AP step is 16-byte aligned
    # (walrus `s3_lw_dual_fp8_restrictions` requires step % 16 == 0)
    W567_d = nc.dram_tensor("W567", [128, 4 * 256], fp8, kind="ExternalInput")
    b34_d = nc.dram_tensor("b34", [128, 4], f32, kind="ExternalInput")
    b567_d = nc.dram_tensor("b567", [128, 2], f32, kind="ExternalInput")
    sqA_d = nc.dram_tensor("sqA", [128, 72], bf16, kind="ExternalInput")
    sqB_d = nc.dram_tensor("sqB", [128, 72], bf16, kind="ExternalInput")
    crA_d = nc.dram_tensor("crA", [128, 72], bf16, kind="ExternalInput")
    crB_d = nc.dram_tensor("crB", [128, 72], bf16, kind="ExternalInput")

    n_tiles = n_rows // B
    assert n_rows % B == 0
    C = B // P  # 128-row chunks per tile

    with TileContext(nc) as tc:
        with (
            tc.tile_pool(name="consts", bufs=1) as consts,
            tc.tile_pool(name="acts", bufs=4) as acts,
            tc.tile_pool(name="pmm", bufs=2, space="PSUM") as pmm,
            tc.tile_pool(name="psd", bufs=2, space="PSUM") as psd,
            tc.tile_pool(name="ppq", bufs=2, space="PSUM") as ppq,
        ):
            identf = consts.tile([128, 128], f32, tag="identf")
            make_identity(nc, identf)
            w12 = consts.tile([73, 256], bf16, tag="w12")
            nc.sync.dma_start(out=w12, in_=w12_d[:])
            W34 = consts.tile([128, 2, 512], fp8, tag="W34")
            nc.sync.dma_start(out=W34, in_=W34_d[:].rearrange("p (k o) -> p k o", k=2))
            W567 = consts.tile([128, 4, 256], fp8, tag="W567")
            nc.sync.dma_start(
                out=W567, in_=W567_d[:].rearrange("p (k o) -> p k o", k=4)
            )
            b34 = consts.tile([128, 4], f32, tag="b34")
            nc.sync.dma_start(out=b34, in_=b34_d[:])
            b567 = consts.tile([128, 2], f32, tag="b567")
            nc.sync.dma_start(out=b567, in_=b567_d[:])
            sqA = consts.tile([128, 72], bf16, tag="sqA")
            nc.sync.dma_start(out=sqA, in_=sqA_d[:])
            sqB = consts.tile([128, 72], bf16, tag="sqB")
            nc.sync.dma_start(out=sqB, in_=sqB_d[:])
            crA = consts.tile([128, 72], bf16, tag="crA")
            nc.sync.dma_start(out=crA, in_=crA_d[:])
            crB = consts.tile([128, 72], bf16, tag="crB")
            nc.sync.dma_start(out=crB, in_=crB_d[:])

            # xT ring: row 72 is a constant ones-row (bias input for LA).
            NXT = 6
            xt_bufs = []
            for i in range(NXT):
                t_ = consts.tile([73, B], bf16, tag=f"xT{i}")
                # rows 64:72 are re-written by the per-tile DMA; row 72
                # stays 1.0 (engine ops must start at a multiple of 32)
                nc.vector.memset(t_[64:73, :], 1.0)
                xt_bufs.append(t_)
            # f ring: rows 72:74 of chunk 1 are ones-rows that pick up the
            # (1 + |c_j|^2) value+residual weight rows; 74:128 stay zero.
            NF = 4
            f_bufs = []
            for i in range(NF):
                t_ = consts.tile([128, 2, B], bf16, tag=f"f{i}")
                # rows 64:72 are re-written by the per-tile L7m1 epilogue
                nc.vector.memset(t_[64:128, 1, :], 0.0)
                nc.vector.memset(t_[64:74, 1, :], 1.0)
                f_bufs.append(t_)

            xT_r = xT_d[:].rearrange("j (t b) -> t j b", b=B)
            q_r = q_d[:].rearrange("(t s p) j -> t p s j", p=P, s=C)

            def dma_in(t):
                nc.sync.dma_start(out=xt_bufs[t % NXT][0:72, :], in_=xT_r[t])

            psA = {}   # t -> LA psum tile
            psB = {}   # t -> (psB, psB2)
            psF = {}   # t -> LC psum tile
            hAs = {}   # t -> hA
            hBs = {}   # t -> hB
            fg = {}    # t -> (f, g)
            sds = {}   # t -> sdT psum tile
            nomT = {}  # t -> nomT tile
            pqs = {}   # t -> pq psum tile

            def laA(t):
                xt = xt_bufs[t % NXT]
                ps = pmm.tile([128, 2, B], f32, tag="mm")
                nc.tensor.matmul(ps[:, 0, :], w12[:, 0:128], xt, start=True, stop=True)
                nc.tensor.matmul(ps[:, 1, :], w12[:, 128:256], xt, start=True, stop=True)
                psA[t] = ps

            def epA(t):
                ps = psA.pop(t)
                hA = acts.tile([128, 2, B], fp8, tag="hA")
                nc.scalar.activation(out=hA[:, 0, :], in_=ps[:, 0, :], func=AF.Relu)
                nc.vector.tensor_scalar_max(hA[:, 1, :], ps[:, 1, :], 0.0)
                hAs[t] = hA

            def lbB(t, half):
                hA = hAs[t]
                ps = pmm.tile([128, 2, B], f32, tag="mm")
                for m in (0, 1):
                    mm = 2 * half + m
                    nc.tensor.matmul(
                        ps[:, m, :], W34[:, 0:2, 128 * mm : 128 * (mm + 1)],
                        hA[:, 0:2, :], start=True, stop=True, perf_mode=DR,
                    )
                if half == 0:
                    psB[t] = [ps]
                else:
                    psB[t].append(ps)
                    hAs.pop(t)

            def epB(t):
                ps1, ps2 = psB.pop(t)
                hB = acts.tile([128, 4, B], fp8, tag="hB")
                nc.scalar.activation(
                    out=hB[:, 0, :], in_=ps1[:, 0, :], func=AF.Relu, bias=b34[:, 0:1]
                )
                nc.scalar.activation(
                    out=hB[:, 1, :], in_=ps1[:, 1, :], func=AF.Relu, bias=b34[:, 1:2]
                )
                nc.vector.tensor_scalar(
                    out=hB[:, 2, :], in0=ps2[:, 0, :],
                    scalar1=b34[:, 2:3], scalar2=0.0, op0=ALU.add, op1=ALU.max,
                )
                nc.vector.tensor_scalar(
                    out=hB[:, 3, :], in0=ps2[:, 1, :],
                    scalar1=b34[:, 3:4], scalar2=0.0, op0=ALU.add, op1=ALU.max,
                )
                hBs[t] = hB

            def lcC(t):
                hB = hBs.pop(t)
                ps = pmm.tile([128, 2, B], f32, tag="mm")
                for m in (0, 1):
                    nc.tensor.matmul(
                        ps[:, m, :], W567[:, 0:2, 128 * m : 128 * (m + 1)],
                        hB[:, 0:2, :], start=True, stop=False, perf_mode=DR,
                    )
                    nc.tensor.matmul(
                        ps[:, m, :], W567[:, 2:4, 128 * m : 128 * (m + 1)],
                        hB[:, 2:4, :], start=False, stop=True, perf_mode=DR,
                    )
                psF[t] = ps

            def epC(t):
                ps = psF.pop(t)
                f = f_bufs[t % NF]
                nc.scalar.activation(
                    out=f[:, 0, :], in_=ps[:, 0, :], func=AF.Identity,
                    bias=b567[:, 0:1],
                )
                nc.scalar.activation(
                    out=f[0:72, 1, :], in_=ps[0:72, 1, :], func=AF.Identity,
                    bias=b567[0:72, 1:2],
                )
                g = acts.tile([128, 2, B], bf16, tag="g")
                fg[t] = (f, g)

            def gsq(t):
                f, g = fg[t]
                nc.vector.tensor_mul(g[:, 0, :], f[:, 0, :], f[:, 0, :])
                nc.scalar.square(g[:, 1, :], f[:, 1, :])

            def distmm(t):
                f, g = fg.pop(t)
                sdT = psd.tile([72, B], f32, tag="sd")
                nc.tensor.matmul(sdT, sqA, g[:, 0, :], start=True, stop=False)
                nc.tensor.matmul(sdT, sqB, g[:, 1, :], start=False, stop=False)
                nc.tensor.matmul(sdT, crA, f[:, 0, :], start=False, stop=False)
                nc.tensor.matmul(sdT, crB, f[:, 1, :], start=False, stop=True)
                sds[t] = sdT

            def recip(t):
                sdT = sds.pop(t)
                nt = acts.tile([72, B], f32, tag="nomT")
                nc.vector.reciprocal_approx_fast(out=nt, in_=sdT)
                nomT[t] = nt

            def do_pq(t):
                nt = nomT.pop(t)
                pq = ppq.tile([P, C, 72], f32, tag="pq")
                for s in range(C):
                    nc.tensor.transpose(
                        pq[:, s, :], nt[:, P * s : P * (s + 1)], identf[:72, :72]
                    )
                pqs[t] = pq

            def stage3(t):
                pq = pqs.pop(t)
                rs4 = acts.tile([P, C], f32, tag="rs4")
                nc.vector.reduce_sum(rs4, pq, axis=AX.X)
                rr4 = acts.tile([P, C], f32, tag="rr4")
                nc.vector.reciprocal(rr4, rs4)
                rr_b = bass.AP(
                    tensor=rr4.tensor,
                    offset=rr4.offset,
                    ap=[rr4.ap[0], rr4.ap[1], [0, 72]],
                )
                qt = acts.tile([P, C, 72], bf16, tag="qt")
                nc.vector.tensor_tensor(out=qt, in0=pq, in1=rr_b, op=ALU.mult)
                nc.sync.dma_start(out=q_r[t], in_=qt)

            # Pair-interleaved software pipeline: tiles (2i, 2i+1) run their
            # MLP while the previous pair's dist/normalize stages fill the
            # gaps, keeping every engine busy and the PE p-state warm.
            assert n_tiles % 2 == 0
            for t in range(min(4, n_tiles)):
                dma_in(t)
            for i in range(n_tiles // 2):
                t0, t1 = 2 * i, 2 * i + 1
                u0, u1 = t0 - 2, t1 - 2  # previous pair
                for t in (t0 + 4, t1 + 4):
                    if t < n_tiles:
                        dma_in(t)
                laA(t0)
                laA(t1)
                epA(t0)
                if u0 >= 0:
                    distmm(u0)
                epA(t1)
                if u0 >= 0:
                    distmm(u1)
                    recip(u0)
                    recip(u1)
                lbB(t0, 0)
                lbB(t0, 1)
                if u0 >= 0:
                    do_pq(u0)
                epB(t0)
                lbB(t1, 0)
                lbB(t1, 1)
                if u0 >= 0:
                    do_pq(u1)
                epB(t1)
                lcC(t0)
                lcC(t1)
                epC(t0)
                epC(t1)
                if u0 >= 0:
                    stage3(u0)
                    stage3(u1)
                gsq(t0)
                gsq(t1)
            # drain last pair
            for t in (n_tiles - 2, n_tiles - 1):
                distmm(t)
                recip(t)
                do_pq(t)
                stage3(t)

    nc.compile()
    return nc


def _prep_consts(ws, bs, center):
    """Host-side composition + marshalling of the replicated weights."""
    import ml_dtypes

    bf = ml_dtypes.bfloat16
    f8 = ml_dtypes.float8_e4m3fn
    w = [np.asarray(x, dtype=np.float64) for x in ws]
    b = [np.asarray(x, dtype=np.float64) for x in bs]
    c = np.asarray(center, dtype=np.float64)

    w12 = w[0] @ w[1]
    b12 = b[0] @ w[1] + b[1]
    W34 = w[2] @ w[3]
    b34 = b[2] @ w[3] + b[3]
    W567 = w[4] @ w[5] @ w[6]
    b567 = (b[4] @ w[5] + b[5]) @ w[6] + b[6]

    consts = {}
    consts["w12"] = np.ascontiguousarray(
        2.0 * np.vstack([w12, b12[None, :]])
    ).astype(bf)
    # k-major fp8 layouts: W[p, k*dout + o] = scale * W[k*128 + p, o]
    consts["W34"] = np.ascontiguousarray(
        (4.0 * W34).reshape(2, 128, 512).transpose(1, 0, 2).reshape(128, 1024)
    ).astype(f8)
    W567p = np.concatenate([8.0 * W567, np.zeros((512, 56))], axis=1)  # pad 200->256
    consts["W567"] = np.ascontiguousarray(
        W567p.reshape(4, 128, 256).transpose(1, 0, 2).reshape(128, 1024)
    ).astype(f8)
    consts["b34"] = np.ascontiguousarray(
        (8.0 * b34).reshape(4, 128).T
    ).astype(np.float32)
    bt = np.zeros((128, 2))
    bt[0:128, 0] = 64.0 * b567[0:128]
    bt[0:72, 1] = 64.0 * b567[128:200]
    consts["b567"] = bt.astype(np.float32)

    csq = 1.0 + (c**2).sum(axis=0)  # [72]
    main = csq.astype(bf)
    resid = (csq - main.astype(np.float64)).astype(bf)
    ones_w = np.float64(2.0**-12)  # f' = 64 f, so sum(g')*2^-12 = |f|^2
    sqA = np.full((128, 72), ones_w)
    sqB = np.zeros((128, 72))
    sqB[0:72, :] = ones_w
    sqB[72, :] = main.astype(np.float64)
    sqB[73, :] = resid.astype(np.float64)
    consts["sqA"] = sqA.astype(bf)
    consts["sqB"] = sqB.astype(bf)
    crA = np.ascontiguousarray(-c[0:128, :] / 32.0)  # f' * (-c/32) = -2 f c
    crB = np.zeros((128, 72))
    crB[0:72, :] = -c[128:200, :] / 32.0
    consts["crA"] = crA.astype(bf)
    consts["crB"] = crB.astype(bf)
    return consts


def _make_in_maps(inputs, ws, bs, center):
    import ml_dtypes

    bf = ml_dtypes.bfloat16
    x = np.asarray(inputs)
    n = x.shape[0]
    n_loc = n // N_CORES
    consts = _prep_consts(ws, bs, center)
    xT = np.ascontiguousarray(x.astype(bf).T)  # [72, N]
    in_maps = []
    for cid in range(N_CORES):
        m = {"xT": np.ascontiguousarray(xT[:, cid * n_loc : (cid + 1) * n_loc])}
        m.update(consts)
        in_maps.append(m)
    return in_maps, n_loc


def kernel(
    inputs, w1, b1, w2, b2, w3, b3, w4, b4, w5, b5, w6, b6, w7, b7, center
):
    from concourse.bass_utils import run_bass_kernel_spmd

    in_maps, n_loc = _make_in_maps(
        inputs,
        [w1, w2, w3, w4, w5, w6, w7],
        [b1, b2, b3, b4, b5, b6, b7],
        center,
    )
    if n_loc not in _CACHE:
        _CACHE[n_loc] = _build(n_loc)
    nc = _CACHE[n_loc]
    res = run_bass_kernel_spmd(nc, in_maps, core_ids=list(range(N_CORES)))
    out = np.concatenate(
        [np.asarray(res.results[c]["q"]) for c in range(N_CORES)], axis=0
    )
    return out.astype(np.float32)
